# revision 15
# baseline (speedup 1.0000x reference)
"""Trainium2 Bass kernel for MultiHeadHypergraphAttention.

Problem: queries (4, 1024, 512), keys (4, 4096, 512), incidence (4, 1024, 4096) i32,
torch-Linear Q/K/V/O projections, per-head masked softmax attention.

Sharding (8 cores): batch (4) x head-group (2 groups of 4 heads).
Core c handles batch b = c//2, head group g = c%2 and produces the partial
output projection for its 4 heads; the host sums the two partials per batch.

Device-side layout ("scores transposed"): S^T is computed with nodes on
partitions and edges on the free axis, so the incidence mask (host-transposed
to (nodes, edges)) is applied in its natural layout and attention weights P^T
feed the attn@V matmul directly as the moving operand (V' stationary), which
produces O^T (head dims on partitions) — exactly the orientation the output
projection needs, so no on-chip transposes at all.

Softmax normalization is folded into the output: V is augmented with a
ones-column so attn@V also produces row sums; O^T rows are divided by those
sums (reciprocal broadcast across partitions via a small SBUF->SBUF DMA).
Masked entries become exp(s/8 - 40) ~ 1e-16 ~ 0, matching the -1e9 mask.

All matmuls run in bf16 (1 cycle/row on the PE when the contraction uses all
128 partitions) with f32 PSUM accumulation. The per-head scores matmul only
contracts over d_k=64, which runs at 1.5 cyc/row; instead Q^T is stored
zero-padded per head to the full 128 partitions of its head-pair so every
scores matmul contracts over C=128 at 1.0 cyc/row (the zero half contributes
nothing). Head 0 applies the mask additively in PSUM via a 320-scaled
identity matmul (exp bias -40); heads 1-3 multiply exp(s/8) by the bf16 mask
on the vector engine — balancing PE/ACT/DVE occupancy.
"""

import sys
import os

for _p in ("/opt/trn_rl_repo",):
    if _p not in sys.path and os.path.isdir(_p):
        sys.path.insert(0, _p)

import numpy as np
from contextlib import ExitStack

import concourse.bass as bass
import concourse.mybir as mybir
import concourse.tile as tile
from concourse import bacc
from concourse.bass_utils import run_bass_kernel_spmd

BF16 = mybir.dt.bfloat16
F32 = mybir.dt.float32
I32 = mybir.dt.int32

BS, E, N, D = 4, 1024, 4096, 512
HL = 4                   # heads per core (local)
NCHUNK = N // 128        # 32
ECHUNK = E // 128        # 8
C_MASK = 320.0           # identity scale for additive mask (320 * 0.125 = 40)
EXP_BIAS = -40.0

LAST_EXEC_TIME_NS = None
_CACHED_NC = None


def _build_nc():
    nc = bacc.Bacc("TRN2", target_bir_lowering=False, debug=False, num_devices=8)

    qT_d = nc.dram_tensor("qT", (D, E), F32, kind="ExternalInput").ap()
    kT_d = nc.dram_tensor("kT", (D, N), F32, kind="ExternalInput").ap()
    mT_d = nc.dram_tensor("mT", (N, E), I32, kind="ExternalInput").ap()
    wqT_d = nc.dram_tensor("wqT", (D, 256), F32, kind="ExternalInput").ap()
    wkT_d = nc.dram_tensor("wkT", (D, 256), F32, kind="ExternalInput").ap()
    wvT_d = nc.dram_tensor("wvT", (D + 1, 260), F32, kind="ExternalInput").ap()
    woT_d = nc.dram_tensor("woT", (2, 128, 512), F32, kind="ExternalInput").ap()
    bq_d = nc.dram_tensor("bq2", (2, 128, 1), F32, kind="ExternalInput").ap()
    bk_d = nc.dram_tensor("bk2", (2, 128, 1), F32, kind="ExternalInput").ap()
    bo_d = nc.dram_tensor("bo_row", (1, 512), F32, kind="ExternalInput").ap()
    out_d = nc.dram_tensor("out", (E, 512), F32, kind="ExternalOutput").ap()

    with tile.TileContext(nc) as tc, ExitStack() as ctx:
        persist = ctx.enter_context(tc.tile_pool(name="persist", bufs=1))
        work = ctx.enter_context(tc.tile_pool(name="work", bufs=1))
        ps = ctx.enter_context(tc.tile_pool(name="ps", bufs=1, space="PSUM"))
        dpool = ctx.enter_context(tc.tile_pool(name="dpool", bufs=1, space="DRAM"))

        # ---------------- constants ----------------
        ones_row = persist.tile([1, 128], BF16, tag="ones_row")
        nc.vector.memset(ones_row, 1.0)
        bias_m40 = persist.tile([128, 1], F32, tag="bias_m40")
        nc.vector.memset(bias_m40, EXP_BIAS)
        ident_mask = persist.tile([128, 128], BF16, tag="ident_mask")
        nc.gpsimd.memset(ident_mask, 0.0)
        nc.gpsimd.affine_select(
            out=ident_mask, in_=ident_mask, compare_op=mybir.AluOpType.not_equal,
            fill=C_MASK, base=0, pattern=[[-1, 128]], channel_multiplier=1)

        # ---------------- input loads (DMA casts f32->bf16) ----------------
        def load_cast(tag, dram_ap, shape):
            t = persist.tile(list(shape), BF16, tag=tag, name=tag)
            nc.gpsimd.dma_start(out=t, in_=dram_ap)
            return t

        wqTb = [load_cast(f"wqTb{c}", wqT_d[c * 128:(c + 1) * 128, :], (128, 256))
                for c in range(4)]
        wkTb = [load_cast(f"wkTb{c}", wkT_d[c * 128:(c + 1) * 128, :], (128, 256))
                for c in range(4)]
        wvTb = [load_cast(f"wvTb{c}", wvT_d[c * 128:(c + 1) * 128, :], (128, 260))
                for c in range(4)]
        wv_bias = load_cast("wv_bias", wvT_d[D:D + 1, :], (1, 260))
        woTb = [load_cast(f"woTb{p}", woT_d[p], (128, 512)) for p in range(2)]
        bo_row = load_cast("bo_row", bo_d, (1, 512))
        bqs, bks = [], []
        for p in range(2):
            bq_t = persist.tile([128, 1], F32, tag=f"bq{p}", name=f"bq{p}")
            nc.sync.dma_start(out=bq_t, in_=bq_d[p])
            bqs.append(bq_t)
            bk_t = persist.tile([128, 1], F32, tag=f"bk{p}", name=f"bk{p}")
            nc.sync.dma_start(out=bk_t, in_=bk_d[p])
            bks.append(bk_t)
        qTb = [load_cast(f"qTb{c}", qT_d[c * 128:(c + 1) * 128, :], (128, E))
               for c in range(4)]

        # keys^T and mask^T loads interleaved per 512-node window so arrival
        # order matches the consumption order of the merged
        # projection+attention pipeline below
        kTb = [persist.tile([128, N], BF16, tag=f"kTb{c}", name=f"kTb{c}")
               for c in range(4)]
        Mb = persist.tile([128, NCHUNK * E], BF16, tag="Mb")
        for dw in range(4):
            wsl = slice(dw * 1024, (dw + 1) * 1024)
            for c in range(4):
                nc.gpsimd.dma_start(out=kTb[c][:, wsl],
                                    in_=kT_d[c * 128:(c + 1) * 128, wsl])
            for nn in range(4 * dw, 4 * dw + 4):
                # two node chunks per transfer: (256, E) -> (128, 2E)
                src = mT_d[nn * 256:(nn + 1) * 256, :].rearrange(
                    "(two p) e -> p two e", p=128)
                dst = Mb[:, 2 * nn * E:(2 * nn + 2) * E].rearrange(
                    "p (two e) -> p two e", two=2)
                nc.gpsimd.dma_start(out=dst, in_=src)

        # ---------------- Q projection ----------------
        # Q~T[l] (128, 1024) bf16: rows [64r, 64r+64) = head l's Q^T, rest 0
        # (l = 2p + r), so scores matmuls contract over the full 128
        # partitions (1 cyc/row) against KTs[p].
        QTs = [persist.tile([128, E], BF16, tag=f"QTs{l}", name=f"QTs{l}")
               for l in range(HL)]
        for l in range(HL):
            r = l % 2
            zsl = slice(64 * (1 - r), 64 * (1 - r) + 64)
            nc.gpsimd.memset(QTs[l][zsl, :], 0.0)
        for p in range(2):
            qp = ps.tile([128, E], F32, tag="st", bufs=2, name=f"qp{p}")
            for c in range(4):
                for e2 in range(2):
                    nc.tensor.matmul(
                        qp[:, e2 * 512:(e2 + 1) * 512],
                        wqTb[c][:, p * 128:(p + 1) * 128],
                        qTb[c][:, e2 * 512:(e2 + 1) * 512],
                        start=(c == 0), stop=(c == 3))
            for r in range(2):
                sl = slice(64 * r, 64 * r + 64)
                nc.vector.tensor_scalar_add(QTs[2 * p + r][sl, :], qp[sl, :],
                                            bqs[p][sl, :])

        # ------------- K/V projections merged with attention ---------------
        KTs = [persist.tile([128, N], BF16, tag=f"KTs{p}", name=f"KTs{p}")
               for p in range(2)]
        Vs = persist.tile([128, NCHUNK * 260], BF16, tag="Vs")
        pairN = [persist.tile([128, E], BF16, tag=f"pairN{p}", name=f"pairN{p}")
                 for p in range(2)]
        oTs = {}
        Ps = {}

        def score_part(l, n):
            # scores + exp + mask for (head l, node chunk n) -> P^T in Ps
            p = l // 2
            st = ps.tile([128, E], F32, tag="st", bufs=2, name=f"st{l}_{n}")
            kblk = KTs[p][:, n * 128:(n + 1) * 128]
            if l == 0:
                for e2 in range(2):
                    sl = slice(e2 * 512, (e2 + 1) * 512)
                    nc.tensor.matmul(
                        st[:, sl], ident_mask,
                        Mb[:, n * E + e2 * 512:n * E + (e2 + 1) * 512],
                        start=True, stop=False)
                    nc.tensor.matmul(st[:, sl], kblk, QTs[l][:, sl],
                                     start=False, stop=True)
                P = work.tile([128, E], BF16, tag="P", bufs=5, name=f"P{l}_{n}")
                nc.scalar.activation(P, st, mybir.ActivationFunctionType.Exp,
                                     bias=bias_m40, scale=0.125)
            else:
                for e2 in range(2):
                    sl = slice(e2 * 512, (e2 + 1) * 512)
                    nc.tensor.matmul(st[:, sl], kblk, QTs[l][:, sl],
                                     start=True, stop=True)
                Praw = work.tile([128, E], BF16, tag="Praw", bufs=5,
                                 name=f"Praw{l}_{n}")
                nc.scalar.activation(Praw, st,
                                     mybir.ActivationFunctionType.Exp,
                                     bias=0.0, scale=0.125)
                P = work.tile([128, E], BF16, tag="P", bufs=5, name=f"P{l}_{n}")
                nc.vector.tensor_mul(P, Praw, Mb[:, n * E:(n + 1) * E])
            Ps[(l, n)] = P

        def av_part(l, n):
            # attn @ V' for (head l, node chunk n), accumulating into oTs[l]
            P = Ps.pop((l, n))
            vblk = Vs[:, n * 260 + l * 65:n * 260 + l * 65 + 65]
            for e2 in range(2):
                sl = slice(e2 * 512, (e2 + 1) * 512)
                nc.tensor.matmul(oTs[l][:, sl], vblk, P[:, sl],
                                 start=(n == 0), stop=(n == NCHUNK - 1))

        def normalize(l):
            # divide O'^T head rows by the exp-sum row: recip on ACT via
            # 1/x = exp(-ln x), broadcast across partitions via DRAM bounce
            p, r = l // 2, l % 2
            oT = oTs[l]
            lnx = work.tile([1, E], F32, tag="lnx", bufs=2, name=f"lnx{l}")
            nc.scalar.activation(lnx, oT[64:65, :],
                                 mybir.ActivationFunctionType.Ln,
                                 bias=0.0, scale=1.0)
            recip = work.tile([1, E], F32, tag="recip", bufs=2, name=f"recip{l}")
            nc.scalar.activation(recip, lnx, mybir.ActivationFunctionType.Exp,
                                 bias=0.0, scale=-1.0)
            rec_d = dpool.tile([1, E], F32, tag="rec_d", bufs=2, name=f"rec_d{l}")
            nc.gpsimd.dma_start(out=rec_d, in_=recip)
            recb = work.tile([64, E], F32, tag="recb", bufs=2, name=f"recb{l}")
            nc.gpsimd.dma_start(out=recb, in_=rec_d.to_broadcast((64, E)))
            nc.vector.tensor_mul(pairN[p][64 * r:64 * r + 64, :], oT[0:64, :],
                                 recb)

        for l in (0, 1):
            oTs[l] = ps.tile([65, E], F32, tag="outT", bufs=2, name=f"oT{l}")

        # merged pipeline: per 512-node window, K/V projection work is
        # interleaved between the head-0/1 score chunks so the scalar engine
        # is never starved while the PE runs projections. attn@V lags the
        # scores by one chunk so the PE never waits on exp/mask.
        def proj_k(w, p):
            kp = ps.tile([128, 512], F32, tag="st", bufs=2, name=f"kp{p}_{w}")
            for c in range(4):
                nc.tensor.matmul(
                    kp, wkTb[c][:, p * 128:(p + 1) * 128],
                    kTb[c][:, w * 512:(w + 1) * 512],
                    start=(c == 0), stop=(c == 3))
            nc.vector.tensor_scalar_add(
                KTs[p][:, w * 512:(w + 1) * 512], kp, bks[p])

        def proj_v(n):
            vp = ps.tile([128, 260], F32, tag="st", bufs=2, name=f"vp{n}")
            for c in range(4):
                nc.tensor.matmul(vp, kTb[c][:, n * 128:(n + 1) * 128],
                                 wvTb[c], start=(c == 0), stop=False)
            nc.tensor.matmul(vp, ones_row, wv_bias, start=False, stop=True)
            nc.vector.tensor_copy(Vs[:, n * 260:(n + 1) * 260], vp)

        for w in range(8):
            n0 = 4 * w
            proj_k(w, 0)
            proj_k(w, 1)
            for i, n in enumerate(range(n0, n0 + 4)):
                proj_v(n)
                score_part(0, n)
                if n > 0:
                    av_part(0, n - 1)
                score_part(1, n)
                if n > 0:
                    av_part(1, n - 1)
        av_part(0, NCHUNK - 1)
        av_part(1, NCHUNK - 1)

        # heads 2 and 3, staggered; the previous head's normalization is
        # emitted a few iterations into the next head's stream so it hides
        norm_pending = [0, 1]
        for l in (2, 3):
            oTs[l] = ps.tile([65, E], F32, tag="outT", bufs=2, name=f"oT{l}")
            for n in range(NCHUNK):
                score_part(l, n)
                if n > 0:
                    av_part(l, n - 1)
                if n == 2 and norm_pending:
                    normalize(norm_pending.pop(0))
                if n == 6 and norm_pending:
                    normalize(norm_pending.pop(0))
            av_part(l, NCHUNK - 1)
            norm_pending.append(l)
        while norm_pending:
            normalize(norm_pending.pop(0))

        # ---------------- phase C: output projection (partial) -------------
        for e in range(ECHUNK):
            f = ps.tile([128, 512], F32, tag="outT", bufs=2, name=f"fin{e}")
            nc.tensor.matmul(f, pairN[0][:, e * 128:(e + 1) * 128], woTb[0],
                             start=True, stop=False)
            nc.tensor.matmul(f, pairN[1][:, e * 128:(e + 1) * 128], woTb[1],
                             start=False, stop=False)
            nc.tensor.matmul(f, ones_row, bo_row, start=False, stop=True)
            fo = work.tile([128, 512], F32, tag="fo", bufs=2, name=f"fo{e}")
            nc.vector.tensor_copy(fo, f)
            nc.sync.dma_start(out=out_d[e * 128:(e + 1) * 128, :], in_=fo)

    nc.compile()
    return nc


def _get_nc():
    global _CACHED_NC
    if _CACHED_NC is None:
        _CACHED_NC = _build_nc()
    return _CACHED_NC


def _make_in_maps(queries, keys, incidence_matrix, Wq, bq, Wk, bk, Wv, bv, Wo, bo):
    """Host-side sharding + layout marshalling (transposes only)."""
    queries = np.asarray(queries, dtype=np.float32)
    keys = np.asarray(keys, dtype=np.float32)
    incidence = np.ascontiguousarray(np.asarray(incidence_matrix, dtype=np.int32))
    Wq = np.asarray(Wq, dtype=np.float32)
    Wk = np.asarray(Wk, dtype=np.float32)
    Wv = np.asarray(Wv, dtype=np.float32)
    Wo = np.asarray(Wo, dtype=np.float32)
    bq = np.asarray(bq, dtype=np.float32)
    bk = np.asarray(bk, dtype=np.float32)
    bv = np.asarray(bv, dtype=np.float32)
    bo = np.asarray(bo, dtype=np.float32)

    in_maps = []
    for core in range(8):
        b, g = core // 2, core % 2
        sl = slice(g * 256, (g + 1) * 256)
        wvT = np.zeros((D + 1, 260), np.float32)
        for l in range(HL):
            rows = slice(g * 256 + l * 64, g * 256 + l * 64 + 64)
            wvT[:D, l * 65:l * 65 + 64] = Wv[rows, :].T
            wvT[D, l * 65:l * 65 + 64] = bv[rows]
            wvT[D, l * 65 + 64] = 1.0
        in_maps.append({
            "qT": np.ascontiguousarray(queries[b].T),
            "kT": np.ascontiguousarray(keys[b].T),
            "mT": np.ascontiguousarray(incidence[b].T),
            "wqT": np.ascontiguousarray(Wq[sl, :].T),
            "wkT": np.ascontiguousarray(Wk[sl, :].T),
            "wvT": wvT,
            "woT": np.ascontiguousarray(Wo[:, sl].T).reshape(2, 128, 512).copy(),
            "bq2": bq[sl].reshape(2, 128, 1).copy(),
            "bk2": bk[sl].reshape(2, 128, 1).copy(),
            "bo_row": (bo[None, :] if g == 0 else np.zeros((1, 512), np.float32)).copy(),
        })
    return in_maps


def kernel(**inputs):
    global LAST_EXEC_TIME_NS
    nc = _get_nc()
    in_maps = _make_in_maps(**inputs)
    trace = bool(os.environ.get("BASS_TRACE"))
    if trace:
        _install_ntff_hook()
    res = run_bass_kernel_spmd(nc, in_maps, core_ids=list(range(8)), trace=trace)
    LAST_EXEC_TIME_NS = res.exec_time_ns
    out = np.zeros((BS, E, D), np.float32)
    for b in range(BS):
        out[b] = res.results[2 * b]["out"] + res.results[2 * b + 1]["out"]
    return out


def _install_ntff_hook():
    """Recreate the missing antenv.axon_hooks glue so trace=True captures NTFF."""
    import types
    if "antenv.axon_hooks" in sys.modules:
        return
    try:
        from trn_agent_boot.trn_boot import _ntff_profile_via_ctypes
        hook = _ntff_profile_via_ctypes("/opt/axon/libaxon_pjrt.so")
        m = types.ModuleType("antenv.axon_hooks")
        m.get_axon_ntff_profile_hook = lambda: hook
        m.set_axon_ntff_profile_hook = lambda h: None
        sys.modules["antenv.axon_hooks"] = m
    except Exception:
        pass
